# revision 6
# baseline (speedup 1.0000x reference)
import numpy as np

B, T, D, E, L, MAXP, P = 128, 2048, 256, 32, 8, 168, 168
NCORES = 8
BC = B // NCORES          # 16 batches per core
ROWS = BC * P             # 2688 rows per core
RT = ROWS // 128          # 21 row-tiles of 128
RSL = [512, 512, 512, 512, 512, 128]   # row slices for N<=512 matmuls
SCALE = 1.0 / np.sqrt(D)


def _fold(inp):
    """Host-side parameter folding (f64), then cast to device dtypes."""
    f64 = np.float64
    q1p = (inp["latents"].astype(f64) @ inp["w_lat_q"].astype(f64)
           @ inp["w_ctx_k"].astype(f64).T) * SCALE                    # [L, D]
    W_cv_lo = inp["w_ctx_v"].astype(f64) @ inp["w_lat_out"].astype(f64)
    step_base = (inp["query_pos"][:P] + inp["len_emb"][P][None]).astype(f64)
    q2p = (step_base @ inp["w_step_q"].astype(f64)
           @ inp["w_lat_k"].astype(f64).T) * SCALE                    # [P, D]
    W_lv_so = inp["w_lat_v"].astype(f64) @ inp["w_step_out"].astype(f64)
    shared_pre = step_base @ inp["gate_w1"].astype(f64)[:D] + inp["gate_b1"].astype(f64)
    W1b = inp["gate_w1"].astype(f64)[D:]                              # [D, D]
    w2h = inp["gate_w2"].astype(f64) * 0.5                            # [D, E]
    return q1p, W_cv_lo, q2p, W_lv_so, shared_pre, W1b, w2h


_CACHE = {}


def _build():
    import concourse.bass as bass
    import concourse.mybir as mybir
    from concourse.tile import TileContext
    from concourse.masks import make_identity

    f32 = mybir.dt.float32
    bf16 = mybir.dt.bfloat16
    AF = mybir.ActivationFunctionType
    ALU = mybir.AluOpType
    AX = mybir.AxisListType

    nc = bass.Bass()
    ctx_in = nc.declare_dram_parameter("ctx", [BC, T, D], bf16, isOutput=False)
    q1pT_in = nc.declare_dram_parameter("q1pT", [128, 2 * L], bf16, isOutput=False)
    Wcv_in = nc.declare_dram_parameter("Wcv", [128, 4 * 128], bf16, isOutput=False)
    q2pT_in = nc.declare_dram_parameter("q2pT", [128, 2 * P], bf16, isOutput=False)
    Wlv_in = nc.declare_dram_parameter("Wlv", [128, 2 * D], bf16, isOutput=False)
    W1b_in = nc.declare_dram_parameter("W1b", [128, 4 * 128], bf16, isOutput=False)
    shT_in = nc.declare_dram_parameter("sharedT", [2, 128, ROWS], f32, isOutput=False)
    w2h_in = nc.declare_dram_parameter("w2h", [128, 2 * E], bf16, isOutput=False)
    b2_in = nc.declare_dram_parameter("b2", [E, 1], f32, isOutput=False)
    w_out = nc.declare_dram_parameter("w", [RT, 128, E], f32, isOutput=True)

    with TileContext(nc) as tc:
        with (
            tc.tile_pool(name="consts", bufs=1) as consts,
            tc.tile_pool(name="ctxp", bufs=2) as ctxp,
            tc.tile_pool(name="work", bufs=2) as work,
            tc.tile_pool(name="acc", bufs=1) as acc,
        ):
            # ---- constants ----
            ident = consts.tile([128, 128], bf16)
            make_identity(nc, ident[:])
            q1pT = consts.tile([128, 2 * L], bf16)
            nc.sync.dma_start(out=q1pT[:], in_=q1pT_in[:, :])
            Wcv = consts.tile([128, 4 * 128], bf16)
            nc.sync.dma_start(out=Wcv[:], in_=Wcv_in[:, :])
            q2pT = consts.tile([128, 2 * P], bf16)
            nc.sync.dma_start(out=q2pT[:], in_=q2pT_in[:, :])
            Wlv = consts.tile([128, 2 * D], bf16)
            nc.sync.dma_start(out=Wlv[:], in_=Wlv_in[:, :])
            W1b = consts.tile([128, 4 * 128], bf16)
            nc.sync.dma_start(out=W1b[:], in_=W1b_in[:, :])
            w2h = consts.tile([128, 2 * E], bf16)
            nc.sync.dma_start(out=w2h[:], in_=w2h_in[:, :])
            b2c = consts.tile([E, 1], f32)
            nc.sync.dma_start(out=b2c[:], in_=b2_in[:, :])
            shT = consts.tile([128, 2 * ROWS], f32)
            nc.sync.dma_start(out=shT[:, 0:ROWS], in_=shT_in[0])
            nc.sync.dma_start(out=shT[:, ROWS:2 * ROWS], in_=shT_in[1])

            # latcT for all batches: [128, (m, b, l)] -> [128, m*BC*L + b*L + l]
            latcT = acc.tile([128, 2 * BC * L], bf16)

            # ================= stage 1: per batch =================
            s1 = tc.tile_pool(name="ps_big", bufs=1, space="PSUM")
            ps_big = s1.__enter__()
            s2p = tc.tile_pool(name="ps_sm", bufs=1, space="PSUM")
            ps_sm = s2p.__enter__()
            for b in range(BC):
                nat = ctxp.tile([128, 16 * D], bf16, tag="nat")
                nc.sync.dma_start(
                    out=nat[:].rearrange("p (t d) -> p t d", d=D),
                    in_=ctx_in[b].rearrange("(t p) d -> p t d", p=128))
                ctT = ctxp.tile([128, 2 * T], bf16, tag="ctT")
                nc.sync.dma_start_transpose(ctT[:, 0:T], ctx_in[b, :, 0:128])
                nc.sync.dma_start_transpose(ctT[:, T:2 * T], ctx_in[b, :, 128:256])

                # scores [L, T] (scale folded into q1pT)
                ps_s = ps_big.tile([L, T], f32, tag="scores")
                for j in range(4):
                    for k in range(2):
                        nc.tensor.matmul(
                            ps_s[:, j * 512:(j + 1) * 512],
                            q1pT[:, k * L:(k + 1) * L],
                            ctT[:, k * T + j * 512: k * T + (j + 1) * 512],
                            start=(k == 0), stop=(k == 1))

                # exp + row-sum
                a_sb = work.tile([L, T], bf16, tag="a")
                zc = work.tile([L, 1], f32, tag="z")
                nc.scalar.activation(a_sb[:], ps_s[:], AF.Exp, accum_out=zc[:])
                rz = work.tile([L, 1], f32, tag="rz")
                nc.vector.reciprocal(rz[:], zc[:])

                # aT via PE transpose, 4 tiles per psum bank
                aT = work.tile([128, 16 * L], bf16, tag="aT")
                for g in range(4):
                    ps_t = ps_sm.tile([128, 4 * L], bf16, tag="aTt")
                    for j in range(4):
                        t = g * 4 + j
                        nc.tensor.transpose(
                            ps_t[:, j * L:(j + 1) * L],
                            a_sb[:, t * 128:(t + 1) * 128], ident[0:L, 0:L])
                    eng = nc.vector if g % 2 == 0 else nc.scalar
                    if eng is nc.vector:
                        eng.tensor_copy(aT[:, g * 4 * L:(g + 1) * 4 * L], ps_t[:])
                    else:
                        eng.copy(aT[:, g * 4 * L:(g + 1) * 4 * L], ps_t[:])

                # ac = a @ ctx  -> [L, D], then scale by 1/Z during copy
                ps_ac = ps_sm.tile([L, D], f32, tag="ac")
                for t in range(16):
                    nc.tensor.matmul(
                        ps_ac[:], aT[:, t * L:(t + 1) * L],
                        nat[:, t * D:(t + 1) * D],
                        start=(t == 0), stop=(t == 15))
                ac_sb = work.tile([L, D], bf16, tag="acsb")
                nc.scalar.activation(ac_sb[:], ps_ac[:], AF.Copy, scale=rz[:])

                # acT [D, L]
                ps_acT = ps_sm.tile([128, 2 * L], bf16, tag="acT")
                nc.tensor.transpose(ps_acT[:, 0:L], ac_sb[:, 0:128], ident[0:L, 0:L])
                nc.tensor.transpose(ps_acT[:, L:2 * L], ac_sb[:, 128:256], ident[0:L, 0:L])
                acT = work.tile([128, 2 * L], bf16, tag="acTsb")
                nc.vector.tensor_copy(acT[:], ps_acT[:])

                # latcT[:, m] = sum_k Wcv[k,m]^T @ acT[k]
                for m in range(2):
                    ps_l = ps_sm.tile([128, L], f32, tag="latc")
                    for k in range(2):
                        nc.tensor.matmul(
                            ps_l[:], Wcv[:, (k * 2 + m) * 128:(k * 2 + m + 1) * 128],
                            acT[:, k * L:(k + 1) * L],
                            start=(k == 0), stop=(k == 1))
                    eng = nc.vector if m == 0 else nc.scalar
                    dst = latcT[:, (m * BC + b) * L:(m * BC + b + 1) * L]
                    if eng is nc.vector:
                        eng.tensor_copy(dst, ps_l[:])
                    else:
                        eng.copy(dst, ps_l[:])

            s2p.__exit__(None, None, None)
            s1.__exit__(None, None, None)
            # ================= phase B =================
            pb = tc.tile_pool(name="ps_b", bufs=1, space="PSUM")
            ps_sm = pb.__enter__()
            # s2 softmax -> a2T [L, ROWS]
            a2T = acc.tile([L, ROWS], bf16)
            for m2, pw in ((0, 128), (1, 40)):
                ps_s2 = ps_sm.tile([128, BC * L], f32, tag="s2")
                for k in range(2):
                    for b in range(BC):
                        nc.tensor.matmul(
                            ps_s2[0:pw, b * L:(b + 1) * L],
                            q2pT[:, k * P + m2 * 128: k * P + m2 * 128 + pw],
                            latcT[:, (k * BC + b) * L:(k * BC + b + 1) * L],
                            start=(k == 0), stop=(k == 1))
                ex2 = work.tile([128, BC * L], bf16, tag="ex2")
                nc.scalar.activation(ex2[0:pw], ps_s2[0:pw], AF.Exp)
                sm2 = work.tile([128, BC], f32, tag="sm2")
                nc.vector.tensor_reduce(
                    sm2[0:pw], ex2[0:pw].rearrange("p (b l) -> p b l", l=L),
                    axis=AX.X, op=ALU.add)
                rr2 = work.tile([128, BC], f32, tag="rr2")
                nc.vector.reciprocal(rr2[0:pw], sm2[0:pw])
                a2n = work.tile([128, BC * L], bf16, tag="a2n")
                nc.vector.tensor_tensor(
                    out=a2n[0:pw].rearrange("p (b l) -> p b l", l=L),
                    in0=ex2[0:pw].rearrange("p (b l) -> p b l", l=L),
                    in1=rr2[0:pw].unsqueeze(-1).broadcast_to((pw, BC, L)),
                    op=ALU.mult)
                # transpose to a2T slices
                for b in range(BC):
                    ps_a2t = ps_sm.tile([L, 128], bf16, tag="a2t")
                    nc.tensor.transpose(
                        ps_a2t[:, 0:pw], a2n[0:pw, b * L:(b + 1) * L],
                        ident[0:pw, 0:pw])
                    eng = nc.vector if b % 2 == 0 else nc.scalar
                    dst = a2T[:, b * P + m2 * 128: b * P + m2 * 128 + pw]
                    if eng is nc.vector:
                        eng.tensor_copy(dst, ps_a2t[:, 0:pw])
                    else:
                        eng.copy(dst, ps_a2t[:, 0:pw])

            # v2 [L, D] per batch
            v2 = acc.tile([L, BC * D], bf16)
            for b in range(BC):
                ps_v2 = ps_sm.tile([L, D], f32, tag="v2")
                for k in range(2):
                    nc.tensor.matmul(
                        ps_v2[:], latcT[:, (k * BC + b) * L:(k * BC + b + 1) * L],
                        Wlv[:, k * D:(k + 1) * D],
                        start=(k == 0), stop=(k == 1))
                eng = nc.vector if b % 2 == 0 else nc.scalar
                if eng is nc.vector:
                    eng.tensor_copy(v2[:, b * D:(b + 1) * D], ps_v2[:])
                else:
                    eng.copy(v2[:, b * D:(b + 1) * D], ps_v2[:])

            # stepcT [kd][128, ROWS]
            stepcT = acc.tile([128, 2 * ROWS], bf16)
            for b in range(BC):
                for kd in range(2):
                    ps_st = ps_sm.tile([128, P], f32, tag="st")
                    nc.tensor.matmul(
                        ps_st[:], v2[:, b * D + kd * 128: b * D + (kd + 1) * 128],
                        a2T[:, b * P:(b + 1) * P], start=True, stop=True)
                    eng = nc.vector if (b + kd) % 2 == 0 else nc.scalar
                    dst = stepcT[:, kd * ROWS + b * P: kd * ROWS + (b + 1) * P]
                    if eng is nc.vector:
                        eng.tensor_copy(dst, ps_st[:])
                    else:
                        eng.copy(dst, ps_st[:])

            # preT pieces: h0 = pre (bf16), h1 = pre*erf(pre/sqrt2) (bf16)
            h0 = acc.tile([128, 2 * ROWS], bf16)
            h1 = acc.tile([128, 2 * ROWS], bf16)
            for md in range(2):
                r0 = 0
                for w_ in RSL:
                    ps_pre = ps_sm.tile([128, 512], f32, tag="pre")
                    for kd in range(2):
                        nc.tensor.matmul(
                            ps_pre[:, 0:w_],
                            W1b[:, (kd * 2 + md) * 128:(kd * 2 + md + 1) * 128],
                            stepcT[:, kd * ROWS + r0: kd * ROWS + r0 + w_],
                            start=(kd == 0), stop=(kd == 1))
                    sl = slice(md * ROWS + r0, md * ROWS + r0 + w_)
                    nc.vector.tensor_add(h0[:, sl], ps_pre[:, 0:w_],
                                         shT[:, sl])
                    er = work.tile([128, 512], bf16, tag="er")
                    nc.scalar.activation(er[:, 0:w_], h0[:, sl], AF.Erf,
                                         scale=float(1.0 / np.sqrt(2.0)))
                    nc.vector.tensor_mul(h1[:, sl], h0[:, sl], er[:, 0:w_])
                    r0 += w_

            # logitsT [E, ROWS] bf16 (bias b2 folded)
            lgT = acc.tile([E, ROWS], bf16)
            r0 = 0
            for w_ in RSL:
                ps_lg = ps_sm.tile([E, 512], f32, tag="lg")
                for i, piece in enumerate((h0, h1)):
                    for md in range(2):
                        nc.tensor.matmul(
                            ps_lg[:, 0:w_], w2h[:, md * E:(md + 1) * E],
                            piece[:, md * ROWS + r0: md * ROWS + r0 + w_],
                            start=(i == 0 and md == 0), stop=(i == 1 and md == 1))
                nc.scalar.activation(lgT[:, r0:r0 + w_], ps_lg[:, 0:w_],
                                     AF.Identity, bias=b2c[:])
                r0 += w_

            # transpose logits to [128, RT*E] f32
            lg = acc.tile([128, RT * E], f32)
            for t in range(RT):
                ps_lt = ps_sm.tile([128, E], bf16, tag="lt")
                nc.tensor.transpose(
                    ps_lt[:], lgT[:, t * 128:(t + 1) * 128], ident[0:E, 0:E])
                eng = nc.vector if t % 2 == 0 else nc.scalar
                if eng is nc.vector:
                    eng.tensor_copy(lg[:, t * E:(t + 1) * E], ps_lt[:])
                else:
                    eng.copy(lg[:, t * E:(t + 1) * E], ps_lt[:])

            # top-2 masked softmax over E (axis X on [128, RT, E])
            lg3 = lg[:].rearrange("p (t e) -> p t e", e=E)
            m1 = work.tile([128, RT], f32, tag="m1")
            nc.vector.tensor_reduce(m1[:], lg3, axis=AX.X, op=ALU.max)
            dd = work.tile([128, RT * E], f32, tag="dd")
            dd3 = dd[:].rearrange("p (t e) -> p t e", e=E)
            m1b = m1[:].unsqueeze(-1).broadcast_to((128, RT, E))
            nc.vector.tensor_tensor(out=dd3, in0=lg3, in1=m1b, op=ALU.subtract)
            ee = work.tile([128, RT * E], f32, tag="ee")
            nc.scalar.activation(ee[:], dd[:], AF.Exp)
            eq = work.tile([128, RT * E], f32, tag="eq")
            nc.vector.tensor_scalar(out=eq[:], in0=dd[:], scalar1=0.0,
                                    scalar2=None, op0=ALU.is_ge)
            t2 = work.tile([128, RT * E], f32, tag="t2")
            nc.vector.scalar_tensor_tensor(
                out=t2[:], in0=eq[:], scalar=-1e30, in1=dd[:],
                op0=ALU.mult, op1=ALU.add)
            m2 = work.tile([128, RT], f32, tag="m2")
            t23 = t2[:].rearrange("p (t e) -> p t e", e=E)
            nc.vector.tensor_reduce(m2[:], t23, axis=AX.X, op=ALU.max)
            mk = work.tile([128, RT * E], f32, tag="mk")
            m2b = m2[:].unsqueeze(-1).broadcast_to((128, RT, E))
            mk3 = mk[:].rearrange("p (t e) -> p t e", e=E)
            nc.vector.tensor_tensor(out=mk3, in0=dd3, in1=m2b, op=ALU.is_ge)
            wu = work.tile([128, RT * E], f32, tag="wu")
            nc.vector.tensor_mul(wu[:], ee[:], mk[:])
            zz = work.tile([128, RT], f32, tag="zz")
            wu3 = wu[:].rearrange("p (t e) -> p t e", e=E)
            nc.vector.tensor_reduce(zz[:], wu3, axis=AX.X, op=ALU.add)
            rzz = work.tile([128, RT], f32, tag="rzz")
            nc.vector.reciprocal(rzz[:], zz[:])
            wf = work.tile([128, RT * E], f32, tag="wf")
            rzb = rzz[:].unsqueeze(-1).broadcast_to((128, RT, E))
            wf3 = wf[:].rearrange("p (t e) -> p t e", e=E)
            nc.vector.tensor_tensor(out=wf3, in0=wu3, in1=rzb, op=ALU.mult)
            nc.sync.dma_start(
                out=w_out[:].rearrange("t p e -> p t e"),
                in_=wf[:].rearrange("p (t e) -> p t e", e=E))
            pb.__exit__(None, None, None)

    bass._bass_rust.generate_event_semaphores(nc)
    return nc


def _prep_maps(inp, q1p, W_cv_lo, q2p, W_lv_so, shared_pre, W1b, w2h):
    import ml_dtypes
    bf = ml_dtypes.bfloat16
    f32 = np.float32

    ctx = np.asarray(inp["ctx_embed"], dtype=f32).astype(bf)   # [B, T, D]
    q1pT = np.zeros((128, 2 * L), dtype=bf)
    for k in range(2):
        q1pT[:, k * L:(k + 1) * L] = q1p.astype(f32).T[k * 128:(k + 1) * 128]
    Wcv = np.zeros((128, 4 * 128), dtype=bf)
    W1bq = np.zeros((128, 4 * 128), dtype=bf)
    for k in range(2):
        for m in range(2):
            Wcv[:, (k * 2 + m) * 128:(k * 2 + m + 1) * 128] = \
                W_cv_lo.astype(f32)[k * 128:(k + 1) * 128, m * 128:(m + 1) * 128]
            W1bq[:, (k * 2 + m) * 128:(k * 2 + m + 1) * 128] = \
                W1b.astype(f32)[k * 128:(k + 1) * 128, m * 128:(m + 1) * 128]
    q2pT = np.zeros((128, 2 * P), dtype=bf)
    for k in range(2):
        q2pT[:, k * P:(k + 1) * P] = q2p.astype(f32).T[k * 128:(k + 1) * 128]
    Wlv = np.zeros((128, 2 * D), dtype=bf)
    for k in range(2):
        Wlv[:, k * D:(k + 1) * D] = W_lv_so.astype(f32)[k * 128:(k + 1) * 128]
    shT = np.zeros((2, 128, ROWS), dtype=f32)
    spT = shared_pre.astype(f32).T                              # [D, P]
    for m in range(2):
        shT[m] = np.tile(spT[m * 128:(m + 1) * 128], (1, BC))
    w2hq = np.zeros((128, 2 * E), dtype=bf)
    for k in range(2):
        w2hq[:, k * E:(k + 1) * E] = w2h.astype(f32)[k * 128:(k + 1) * 128]
    b2 = np.asarray(inp["gate_b2"], dtype=f32).reshape(E, 1)

    shared = dict(q1pT=q1pT, Wcv=Wcv, q2pT=q2pT, Wlv=Wlv, W1b=W1bq,
                  sharedT=shT, w2h=w2hq, b2=b2)
    maps = []
    for c in range(NCORES):
        m = dict(shared)
        m["ctx"] = np.ascontiguousarray(ctx[c * BC:(c + 1) * BC])
        maps.append(m)
    return maps


def _host_reference(inp):
    """f32 numpy fallback (same math as reference)."""
    f32 = np.float32
    q1p, W_cv_lo, q2p, W_lv_so, shared_pre, W1b, w2h = _fold(inp)
    Pv = int(inp["pred_len"])
    f64 = np.float64
    ctx = inp["ctx_embed"].astype(f64)
    sb = (inp["query_pos"][:Pv] + inp["len_emb"][Pv][None]).astype(f64)
    q1 = (inp["latents"].astype(f64) @ inp["w_lat_q"].astype(f64)
          @ inp["w_ctx_k"].astype(f64).T) * SCALE
    q2 = (sb @ inp["w_step_q"].astype(f64) @ inp["w_lat_k"].astype(f64).T) * SCALE
    shp = sb @ inp["gate_w1"].astype(f64)[:D] + inp["gate_b1"].astype(f64)
    s = np.einsum("ld,btd->blt", q1, ctx)
    a = np.exp(s - s.max(2, keepdims=True)); a /= a.sum(2, keepdims=True)
    latc = np.einsum("blt,btd->bld", a, ctx) @ W_cv_lo
    s2 = np.einsum("pd,bld->bpl", q2, latc)
    a2 = np.exp(s2 - s2.max(2, keepdims=True)); a2 /= a2.sum(2, keepdims=True)
    stepc = a2 @ (latc @ W_lv_so)
    pre = shp[None] + stepc @ W1b
    from scipy.special import erf
    h = pre * 0.5 * (1.0 + erf(pre / np.sqrt(2.0)))
    logits = h @ inp["gate_w2"].astype(f64) + inp["gate_b2"].astype(f64)
    srt = np.sort(logits, 2); kth = srt[:, :, -2:-1]
    ex = np.where(logits >= kth, np.exp(logits - logits.max(2, keepdims=True)), 0.0)
    w = ex / ex.sum(2, keepdims=True)
    out = np.einsum("bpe,epd->bpd", w, inp["query_experts"][:, :Pv, :].astype(f64))
    return out.astype(f32)


def kernel(ctx_embed, query_experts, query_pos, len_emb, latents,
           w_lat_q, w_ctx_k, w_ctx_v, w_lat_out,
           w_step_q, w_lat_k, w_lat_v, w_step_out,
           gate_w1, gate_b1, gate_w2, gate_b2, pred_len):
    inp = dict(ctx_embed=np.asarray(ctx_embed), query_experts=np.asarray(query_experts),
               query_pos=np.asarray(query_pos), len_emb=np.asarray(len_emb),
               latents=np.asarray(latents), w_lat_q=np.asarray(w_lat_q),
               w_ctx_k=np.asarray(w_ctx_k), w_ctx_v=np.asarray(w_ctx_v),
               w_lat_out=np.asarray(w_lat_out), w_step_q=np.asarray(w_step_q),
               w_lat_k=np.asarray(w_lat_k), w_lat_v=np.asarray(w_lat_v),
               w_step_out=np.asarray(w_step_out), gate_w1=np.asarray(gate_w1),
               gate_b1=np.asarray(gate_b1), gate_w2=np.asarray(gate_w2),
               gate_b2=np.asarray(gate_b2), pred_len=pred_len)
    if int(inp["pred_len"]) != P:
        return _host_reference(inp)
    try:
        from concourse.bass_utils import run_bass_kernel_spmd
        folded = _fold(inp)
        maps = _prep_maps(inp, *folded)
        if "nc" not in _CACHE:
            _CACHE["nc"] = _build()
        res = run_bass_kernel_spmd(_CACHE["nc"], maps, list(range(NCORES)))
        w = np.stack([np.asarray(res.results[c]["w"]).reshape(ROWS, E)
                      for c in range(NCORES)])            # [8, 2688, 32]
        w = w.reshape(B, P, E).astype(np.float32)
        qe = np.asarray(inp["query_experts"], dtype=np.float32)[:, :P, :]
        out = np.matmul(w.transpose(1, 0, 2), qe.transpose(1, 0, 2))
        return np.ascontiguousarray(out.transpose(1, 0, 2)).astype(np.float32)
    except Exception:
        import traceback
        traceback.print_exc()
        return _host_reference(inp)
